# revision 52
# baseline (speedup 1.0000x reference)
"""ExclusiveSelfAttention TRN2 kernel v2: head-sharded tensor parallel, 8 cores.

16 heads / 8 cores = 2 heads (128 channels) per core. Per core:
  - q/k projections in [e, s] layout (weights stationary, x moving, N=512)
  - v projection directly in NATURAL [s, e] layout (x stationary, w moving)
    so no DRAM-round-trip transpose is needed; bias folded in via a
    row-replicated bias tile (tensor_tensor add during the PSUM drain).
  - attention in transposed form: scores^T[j, i] per 128-j tile, softmax-exp
    on ACT reading PSUM directly (scale=1/8 fused), fp16 everywhere on chip.
  - PV with exp STATIONARY and v moving (out [i-block, 65] at N=65), which
    charges ~half the PE rows of the v-stationary form; a ones column rides
    in v to produce sumexp per position in the same matmuls.
  - per-position Gram-Schmidt exclusion entirely with per-partition scalars
    (i on partitions): reductions along the free dim on DVE, no PE
    broadcasts needed. EPS dropped: vv in [33, 180] makes it irrelevant.
  - o_f transposed back [i,e]->[e,i] via PE transpose (identity passed as an
    input), then the partial out-projection; partials fp16, summed on host.

PSUM discipline (device-verified): a bank supports only ONE open matmul
accumulation group at a time and groups must be contiguous runs, so each
PV (h, i-block) is an unbroken 16-matmul burst, chained with explicit deps
so the tile scheduler cannot interleave same-bank groups. Bank budget (8):
scores ping-pong 2x[128,1024] (4) + pv h0/h1 (2) + outproj/transpose/proj
scratch (2).

Software-pipelined emission: rounds of (batch, i-quarter); per score-tile
the PE stream carries "filler" quanta (projections of the other batch, PV
bursts and out-projection of the previous round) via a slotted work queue,
so the PE stays fed while ACT runs the exp stream (the ~133us ACT exp
budget is the binding engine). The kernel tail routes PSUM drains to ACT,
which is idle once the exp stream ends.
"""

import sys

if '/opt/trn_rl_repo' not in sys.path:
    sys.path.insert(0, '/opt/trn_rl_repo')

import numpy as np
import ml_dtypes

import concourse.bass as bass
import concourse.mybir as mybir
import concourse.tile as tile
from concourse.bass_utils import run_bass_kernel_spmd

F32 = mybir.dt.float32
F16 = mybir.dt.float16
AF = mybir.ActivationFunctionType
ALU = mybir.AluOpType

B, S, D = 2, 2048, 1024
BS = B * S                    # 4096 combined (b, s) rows
E_LOC = 128                   # channels per core (2 heads x 64)
N_CORES = 8
INV_SQRT_HD = 0.125
NT = 32                       # 128-row sequence tiles
NR = 8                        # rounds: (b, i-quarter), i extent 512 each
IS = 512

_ENGINE_TO_NC = {"PE": "tensor", "DVE": "vector", "Activation": "scalar",
                 "Pool": "gpsimd", "SP": "sync"}


def _make_nop(nc, engine):
    eng = getattr(nc, _ENGINE_TO_NC[str(engine).split(".")[-1]])
    r = eng.nop(nofuse=True, hint="waitsplit")
    ins = r.ins if hasattr(r, "ins") else r
    for blk in nc.main_func.blocks:
        insns = blk.instructions
        for i, x in enumerate(insns):
            if x.name == ins.name:
                del insns[i]
                blk.instructions = insns
                return ins
    raise RuntimeError("freshly created nop not found")


def split_waits(nc, limit=1):
    """Walrus codegen only encodes one sync-wait per instruction here; move
    excess waits onto preceding same-engine NOPs (same-engine program order
    makes this semantics-preserving)."""
    for blk in nc.main_func.blocks:
        ins_list = blk.instructions
        out, changed = [], False
        for ins in ins_list:
            si = ins.sync_info
            if si is not None and len(si.on_wait) > limit:
                waits = list(si.on_wait)
                extra, keep = waits[:-limit], waits[-limit:]
                for w in extra:
                    nop = _make_nop(nc, ins.engine)
                    nop.sync_info = mybir.SyncInfo(on_wait=[w], on_update=[])
                    out.append(nop)
                ins.sync_info = mybir.SyncInfo(on_wait=keep, on_update=list(si.on_update))
                changed = True
            out.append(ins)
        if changed:
            blk.instructions = out


def build_program():
    nc = bass.Bass()

    xT_d = nc.declare_dram_parameter("xT", [D, BS], F16, isOutput=False)
    wqT_d = nc.declare_dram_parameter("wqT", [D, E_LOC], F16, isOutput=False)
    wkT_d = nc.declare_dram_parameter("wkT", [D, E_LOC], F16, isOutput=False)
    wvT_d = nc.declare_dram_parameter("wvT", [D, E_LOC], F16, isOutput=False)
    bq_d = nc.declare_dram_parameter("bq", [E_LOC], F32, isOutput=False)
    bk_d = nc.declare_dram_parameter("bk", [E_LOC], F32, isOutput=False)
    bvr_d = nc.declare_dram_parameter("bvr", [128, E_LOC], F16, isOutput=False)
    id_d = nc.declare_dram_parameter("ident", [128, 128], F16, isOutput=False)
    woT_d = nc.declare_dram_parameter("woT", [E_LOC, D], F16, isOutput=False)
    part_d = nc.declare_dram_parameter("partial", [BS, D], F16, isOutput=True)

    with tile.TileContext(nc) as tc:
        import contextlib
        with contextlib.ExitStack() as ctx:
            const = ctx.enter_context(tc.tile_pool(name="const", bufs=1))
            xpool = ctx.enter_context(tc.tile_pool(name="xpool", bufs=1))
            persist = ctx.enter_context(tc.tile_pool(name="persist", bufs=1))
            etp = ctx.enter_context(tc.tile_pool(name="etp", bufs=26))
            ofp = ctx.enter_context(tc.tile_pool(name="ofp", bufs=24))
            oftp = ctx.enter_context(tc.tile_pool(name="oftp", bufs=4))
            ystp = ctx.enter_context(tc.tile_pool(name="ystp", bufs=4))
            scrp = ctx.enter_context(tc.tile_pool(name="scrp", bufs=2))
            ovp = ctx.enter_context(tc.tile_pool(name="ovp", bufs=2))
            alp = ctx.enter_context(tc.tile_pool(name="alp", bufs=4))
            rzp = ctx.enter_context(tc.tile_pool(name="rzp", bufs=4))
            avrp = ctx.enter_context(tc.tile_pool(name="avrp", bufs=8))
            vvrp = ctx.enter_context(tc.tile_pool(name="vvrp", bufs=2))
            ps_sc = ctx.enter_context(tc.tile_pool(name="ps_sc", bufs=2, space="PSUM"))
            ps_pv = ctx.enter_context(tc.tile_pool(name="ps_pv", bufs=2, space="PSUM"))
            ps_op = ctx.enter_context(tc.tile_pool(name="ps_op", bufs=2, space="PSUM"))

            # ---- constants (DMA order tuned for early k-projection) ----
            xt = xpool.tile([128, 8, BS], F16, tag="xt")

            def dma_x(c, kts=slice(0, 8)):
                nc.sync.dma_start(
                    out=xt[:, kts, c * 512:(c + 1) * 512],
                    in_=xT_d[:, c * 512:(c + 1) * 512].rearrange(
                        "(kt p) s -> p kt s", kt=8)[:, kts, :])

            def dma_x_cols(c0, c1):
                nc.sync.dma_start(
                    out=xt[:, :, c0:c1],
                    in_=xT_d[:, c0:c1].rearrange("(kt p) s -> p kt s", kt=8))

            # DMA order: the critical path to the first exp is
            # ident -> wk -> x[0:256] -> wq -> x[256:512] -> biases
            id_sb = const.tile([128, 128], F16, tag="ident")
            nc.sync.dma_start(out=id_sb, in_=id_d[:, :])
            wk_sb = const.tile([128, 8, E_LOC], F16, tag="wk")
            nc.sync.dma_start(out=wk_sb, in_=wkT_d[:, :].rearrange(
                "(kt p) e -> p kt e", kt=8))
            dma_x_cols(0, 256)
            wq_sb = const.tile([128, 8, E_LOC], F16, tag="wq")
            nc.sync.dma_start(out=wq_sb, in_=wqT_d[:, :].rearrange(
                "(kt p) e -> p kt e", kt=8))
            dma_x_cols(256, 512)
            bk_sb = const.tile([128, 1], F32, tag="bk")
            nc.sync.dma_start(out=bk_sb, in_=bk_d[:].rearrange("(p one) -> p one", one=1))
            bq_sb = const.tile([128, 1], F32, tag="bq")
            nc.sync.dma_start(out=bq_sb, in_=bq_d[:].rearrange("(p one) -> p one", one=1))
            wv_sb = const.tile([128, 8, E_LOC], F16, tag="wv")
            nc.sync.dma_start(out=wv_sb, in_=wvT_d[:, :].rearrange(
                "(kt p) e -> p kt e", kt=8))
            bvr_sb = const.tile([128, E_LOC], F16, tag="bvr")
            nc.sync.dma_start(out=bvr_sb, in_=bvr_d[:, :])
            dma_x(1)
            wo_sb = const.tile([128, D], F16, tag="wo")
            nc.sync.dma_start(out=wo_sb, in_=woT_d[:, :])
            for c in range(2, 8):
                dma_x(c)

            wsb = {"q": wq_sb, "k": wk_sb, "v": wv_sb}

            # ---- persistent activations ----
            qT = persist.tile([128, BS], F16, tag="qT")
            kT = persist.tile([128, BS], F16, tag="kT")
            # vn: cols 0:64 v_h0, 64 ones, 65:129 v_h1, 129 pad
            vn = persist.tile([128, NT, 130], F16, tag="vn")
            vv = persist.tile([128, 2, NT], F32, tag="vv")
            nc.vector.memset(vn[:, :, 64:65], 1.0)

            # ---- projection work quanta ----
            _kq_ps = {}

            def kq_piece(which, c0, c1, kta, ktb):
                """q/k projection of s-columns [c0, c1), kt-tiles [kta, ktb)."""
                key = (which, c0)
                if kta == 0:
                    _kq_ps[key] = ps_op.tile([128, 512], F32, tag="op",
                                             name=f"ps{which}{c0}")
                ps = _kq_ps[key]
                for kt in range(kta, ktb):
                    nc.tensor.matmul(ps[:, 0:c1 - c0], wsb[which][:, kt, :],
                                     xt[:, kt, c0:c1],
                                     start=(kt == 0), stop=(kt == 7))
                if ktb == 8:
                    dst, bias = (qT, bq_sb) if which == "q" else (kT, bk_sb)
                    nc.vector.tensor_scalar(
                        out=dst[:, c0:c1], in0=ps[:, 0:c1 - c0],
                        scalar1=bias, scalar2=None, op0=ALU.add)
                    del _kq_ps[key]

            def kq_half(which, c, half):
                kq_piece(which, c * 512, (c + 1) * 512, 4 * half, 4 * half + 4)

            def v_tile(t):
                """v projection for sequence tile t, into natural layout."""
                ps = ps_op.tile([128, 512], F32, tag="op", name=f"psv{t}")
                for kt in range(8):
                    nc.tensor.matmul(ps[:, 0:128],
                                     xt[:, kt, t * 128:(t + 1) * 128],
                                     wsb["v"][:, kt, :],
                                     start=(kt == 0), stop=(kt == 7))
                nc.vector.tensor_tensor(out=vn[:, t, 0:64], in0=ps[:, 0:64],
                                        in1=bvr_sb[:, 0:64], op=ALU.add)
                nc.vector.tensor_tensor(out=vn[:, t, 65:129], in0=ps[:, 64:128],
                                        in1=bvr_sb[:, 64:128], op=ALU.add)
                for h in range(2):
                    hs = slice(65 * h, 65 * h + 64)
                    scr = scrp.tile([128, 64], F16, tag="scr")
                    nc.vector.scalar_tensor_tensor(
                        out=scr, in0=vn[:, t, hs], scalar=1.0,
                        in1=vn[:, t, hs], op0=ALU.mult, op1=ALU.mult,
                        accum_out=vv[:, h, t:t + 1])

            # ---- attention round machinery ----
            from concourse.tile import add_dep_helper

            def _ins(x):
                return x.ins if hasattr(x, "ins") else x

            def pv_burst(pv, ets, jb, h, m, prev_last):
                """One contiguous 16-matmul accumulation group (h, i-block m).
                PSUM allows only one open accumulation group per bank, so the
                group must run as an unbroken run on its bank; chain it after
                the previous group of the same bank."""
                first = last = None
                for jt in range(16):
                    mvs = vn[:, jb + jt, 0:65] if h == 0 else vn[:, jb + jt, 64:129]
                    r = nc.tensor.matmul(
                        pv[:, 65 * m:65 * m + 65],
                        ets[jt][:, h * 512 + m * 128:h * 512 + (m + 1) * 128],
                        mvs,
                        start=(jt == 0), stop=(jt == 15),
                        skip_group_check=True)
                    if jt == 0:
                        first = r
                    last = r
                if prev_last is not None:
                    add_dep_helper(_ins(first), _ins(prev_last),
                                   reason="psum accumulation groups must not "
                                          "interleave within a bank")
                return last

            ZOFF = {0: 64, 1: 0}   # Z column offset within pv slot
            OOFF = {0: 0, 1: 1}    # o columns offset within pv slot

            def exclusion_m(r, pv, vvr_r, t0, m):
                """Per-i-block Gram-Schmidt exclusion (both heads)."""
                of = ofp.tile([128, 128], F16, tag="of", name=f"of{r}{m}")
                for h in range(2):
                    pvh = pv[h]
                    hs = slice(65 * h, 65 * h + 64)
                    rz = rzp.tile([128, 1], F32, tag="rz")
                    nc.vector.reciprocal(
                        rz, pvh[:, 65 * m + ZOFF[h]:65 * m + ZOFF[h] + 1])
                    ov = ovp.tile([128, 1], F32, tag="ov")
                    scr = scrp.tile([128, 64], F16, tag="scr")
                    nc.vector.scalar_tensor_tensor(
                        out=scr,
                        in0=pvh[:, 65 * m + OOFF[h]:65 * m + OOFF[h] + 64],
                        scalar=1.0,
                        in1=vn[:, t0 + m, hs],
                        op0=ALU.mult, op1=ALU.mult,
                        accum_out=ov)
                    al = alp.tile([128, 1], F32, tag="al")
                    nc.vector.tensor_tensor(out=al, in0=ov,
                                            in1=vvr_r[:, h, m:m + 1],
                                            op=ALU.mult)
                    alr = alp.tile([128, 1], F32, tag="al", name=f"alr{r}{h}{m}")
                    nc.vector.tensor_tensor(out=alr, in0=al, in1=rz, op=ALU.mult)
                    avr = avrp.tile([128, 64], F16, tag="avr")
                    nc.gpsimd.tensor_scalar(
                        out=avr, in0=vn[:, t0 + m, hs],
                        scalar1=alr, scalar2=None, op0=ALU.mult)
                    nc.vector.scalar_tensor_tensor(
                        out=of[:, 64 * h:64 * h + 64],
                        in0=pvh[:, 65 * m + OOFF[h]:65 * m + OOFF[h] + 64],
                        scalar=rz, in1=avr,
                        op0=ALU.mult, op1=ALU.subtract)
                return of

            def outproj_steps_m(r, of, m, use_act=False):
                """3 filler quanta: transpose + 2 half-d out-proj matmuls for
                one i-block. use_act routes PSUM drains to ACT (kernel tail,
                where the exp stream is finished)."""
                oft_box = {}
                cp = nc.scalar.copy if use_act else nc.vector.tensor_copy

                def tr():
                    tp = ps_op.tile([128, 128], F16, tag="op",
                                    padded_shape=[128, 1024],
                                    name=f"tp{r}{m}")
                    nc.tensor.transpose(tp, of, id_sb)
                    oft = oftp.tile([128, 128], F16, tag="oft")
                    cp(oft, tp)
                    oft_box[0] = oft

                def ymm(dh):
                    def f():
                        ps = ps_op.tile([128, 512], F32, tag="op",
                                        name=f"y{r}{m}{dh}")
                        nc.tensor.matmul(ps, oft_box[0],
                                         wo_sb[:, dh * 512:(dh + 1) * 512],
                                         start=True, stop=True)
                        stg = ystp.tile([128, 512], F16, tag="ystg")
                        cp(stg, ps)
                        ig = r * 512 + m * 128
                        nc.sync.dma_start(
                            out=part_d[ig:ig + 128, dh * 512:(dh + 1) * 512],
                            in_=stg)
                    return f

                return [tr, ymm(0), ymm(1)]

            # ---- work queues: (earliest_global_slot, cycles, closure) ----
            # deferq holds work with no deadline (early rounds' out-proj),
            # drained only when the main queue is empty for this slot
            import collections
            workq = collections.deque()
            deferq = collections.deque()
            gslot_box = [0]

            def run_queue(budget):
                while budget > 0:
                    if workq and workq[0][0] <= gslot_box[0]:
                        q = workq
                    elif deferq and deferq[0][0] <= gslot_box[0]:
                        q = deferq
                    else:
                        break
                    _, cyc, f = q.popleft()
                    f()
                    budget -= cyc
                return budget

            def enqueue(earliest, cyc, f, defer=False):
                (deferq if defer else workq).append((earliest, cyc, f))

            def pv_and_excl(r):
                """Enqueue PV bursts of round r into round r+1's slots; after
                both heads' burst of i-block m, its exclusion runs and the
                block's out-projection is enqueued. The last round starts two
                slots early (its PE stream has nothing else left)."""
                b = r // 4
                jb = b * 16
                t0 = r * 4
                last = (r == NR - 1)
                pv = {h: ps_pv.tile([128, 512], F32, tag="pv",
                                    name=f"pv{r}{h}") for h in range(2)}
                ets = ets_of[r]
                state = {0: None, 1: None}
                vvr_box = {}
                base = r * 16 + 14 if last else (r + 1) * 16

                def burst(h, m):
                    def f():
                        if not vvr_box:
                            vvr_r = vvrp.tile([128, 2, 4], F32, tag="vvr")
                            nc.vector.reciprocal(vvr_r, vv[:, :, t0:t0 + 4])
                            vvr_box[0] = vvr_r
                        state[h] = pv_burst(pv[h], ets, jb, h, m, state[h])
                        if h == 1:
                            of = exclusion_m(r, pv, vvr_box[0], t0, m)
                            if m == 3:
                                del ets_of[r]
                            sts = outproj_steps_m(r, of, m, use_act=last)
                            for i, st in enumerate(sts):
                                enqueue(base + 2 * m + 2 + i, 600, st,
                                        defer=(r <= 2))
                    return f

                off = 2 if r == 0 else 0
                for m in range(4):
                    enqueue(base + off + 2 * m, 1040, burst(0, m))
                    enqueue(base + off + 2 * m + 1, 1040, burst(1, m))

            ets_of = {}

            def round_(r, fillers):
                b, qi = divmod(r, 4)
                i0 = b * S + qi * IS
                jb = b * 16
                ets = {}
                ets_of[r] = ets
                for jt in range(16):
                    gslot_box[0] = r * 16 + jt
                    sc = ps_sc.tile([128, 1024], F32, tag="sc")
                    jcol = slice((jb + jt) * 128, (jb + jt + 1) * 128)
                    nc.tensor.matmul(sc[:, 0:512], kT[0:64, jcol],
                                     qT[0:64, i0:i0 + IS],
                                     start=True, stop=True, tile_position=(0, 0))
                    nc.tensor.matmul(sc[:, 512:1024], kT[64:128, jcol],
                                     qT[64:128, i0:i0 + IS],
                                     start=True, stop=True, tile_position=(64, 0))
                    et = etp.tile([128, 1024], F16, tag="et")
                    nc.scalar.activation(et, sc, AF.Exp, bias=0.0,
                                         scale=INV_SQRT_HD)
                    ets[jt] = et
                    budget = 1500
                    for cyc, f in fillers[jt]:
                        f()
                        budget -= cyc
                    run_queue(budget)
                pv_and_excl(r)

            # ---- emission schedule ----
            # warm up the PE p-state during the initial DMA wait: back-to-back
            # junk matmuls (WAW-serialized on one psum slot) keep the PE busy
            # >3us so the first real projection runs at full clock
            warm_ps = ps_op.tile([128, 512], F32, tag="op", name="warm")
            for _ in range(10):
                nc.tensor.matmul(warm_ps[:, 0:128], id_sb, id_sb,
                                 start=True, stop=True)
            # pre-phase: the minimum for scores jt0/jt1 — k of s[0:256] and
            # q of i[0:512]; the rest of k chunk 0 is the first round-0 filler
            kq_piece("k", 0, 256, 0, 8)
            kq_half("q", 0, 0)
            kq_half("q", 0, 1)

            def empty_sched():
                return [[] for _ in range(16)]

            def F_kq(which, c, half):
                return (2048, lambda: kq_half(which, c, half))

            def F_v(t):
                return (1024, lambda: v_tile(t))

            for r in range(NR):
                fill = empty_sched()
                if r == 0:
                    # b0 k chunks (deadline: scores jt 4c), v tiles 4..15
                    # (deadline: pv bursts next round), q(b0,i1) late
                    fill[0] += [(2048, lambda: kq_piece("k", 256, 512, 0, 8))]
                    fill[1] += [F_kq("k", 1, 0)]
                    fill[2] += [F_kq("k", 1, 1)]
                    fill[3] += [F_v(0), F_v(1)]
                    fill[4] += [F_kq("k", 2, 0)]
                    fill[5] += [F_kq("k", 2, 1)]
                    fill[6] += [F_v(2), F_v(3)]
                    fill[7] += [F_v(4), F_v(5)]
                    fill[8] += [F_kq("k", 3, 0)]
                    fill[9] += [F_kq("k", 3, 1)]
                    fill[10] += [F_v(6), F_v(7)]
                    fill[11] += [F_v(8), F_v(9)]
                    fill[12] += [F_v(10), F_v(11)]
                    fill[13] += [F_v(12), F_v(13), F_kq("q", 1, 0)]
                    fill[14] += [F_kq("q", 1, 1)]
                elif r in (1, 2):
                    qc = r + 1          # q(b0, i2) in r1, q(b0, i3) in r2
                    kc = 4 + 2 * (r - 1)  # k(b1) chunks 4,5 in r1; 6,7 in r2
                    if r == 1:
                        fill[0] += [F_v(14)]
                        fill[1] += [F_v(15)]
                    fill[9] += [F_kq("q", qc, 0)]
                    fill[10] += [F_kq("q", qc, 1)]
                    fill[11] += [F_kq("k", kc, 0)]
                    fill[12] += [F_kq("k", kc, 1)]
                    fill[13] += [F_kq("k", kc + 1, 0)]
                    fill[14] += [F_kq("k", kc + 1, 1)]
                elif r == 3:
                    fill[8] += [F_kq("q", 4, 0)]
                    fill[9] += [F_kq("q", 4, 1)]
                    for i, t in enumerate(range(16, 26)):
                        fill[10 + (i * 5) // 10] += [F_v(t)]
                elif r == 4:
                    # remaining b1 v tiles (needed by pv bursts in round 5)
                    fill[8] += [F_kq("q", 5, 0)]
                    fill[9] += [F_kq("q", 5, 1)]
                    for i, t in enumerate(range(26, 32)):
                        fill[10 + (i * 5) // 10] += [F_v(t)]
                elif r in (5, 6):
                    qc = r + 1
                    fill[8] += [F_kq("q", qc, 0)]
                    fill[10] += [F_kq("q", qc, 1)]
                round_(r, fill)

            # tail: drain remaining queued work (last rounds' PV/excl/outproj)
            gslot_box[0] = 10 ** 9
            while workq:
                _, _, f = workq.popleft()
                f()

    split_waits(nc)
    return nc


_CACHE = {}


def kernel(x, wq, bq, wk, bk, wv, bv, wo, bo):
    x = np.ascontiguousarray(np.asarray(x, dtype=np.float32))
    wq, wk, wv, wo = (np.asarray(w, dtype=np.float32) for w in (wq, wk, wv, wo))
    bq, bk, bv, bo = (np.asarray(v, dtype=np.float32) for v in (bq, bk, bv, bo))

    if "nc" not in _CACHE:
        _CACHE["nc"] = build_program()
    nc = _CACHE["nc"]

    xT = np.ascontiguousarray(x.reshape(BS, D).T).astype(np.float16)
    ident = np.eye(128, dtype=np.float16)
    in_maps = []
    for g in range(N_CORES):
        cs = slice(g * E_LOC, (g + 1) * E_LOC)
        in_maps.append({
            "xT": xT,
            "wqT": np.ascontiguousarray(wq[cs, :].T).astype(np.float16),
            "wkT": np.ascontiguousarray(wk[cs, :].T).astype(np.float16),
            "wvT": np.ascontiguousarray(wv[cs, :].T).astype(np.float16),
            "bq": np.ascontiguousarray(bq[cs]),
            "bk": np.ascontiguousarray(bk[cs]),
            "bvr": np.ascontiguousarray(
                np.tile(bv[cs].astype(np.float16)[None, :], (128, 1))),
            "ident": ident,
            "woT": np.ascontiguousarray(wo[:, cs].T).astype(np.float16),
        })

    res = run_bass_kernel_spmd(nc, in_maps, list(range(N_CORES)))
    out = np.zeros((BS, D), np.float32)
    for g in range(N_CORES):
        out += np.asarray(res.results[g]["partial"], np.float32)
    out += bo[None, :]
    return out.reshape(B, S, D)


# revision 53
# speedup vs baseline: 1.0139x; 1.0139x over previous
"""ExclusiveSelfAttention TRN2 kernel v2: head-sharded tensor parallel, 8 cores.

16 heads / 8 cores = 2 heads (128 channels) per core. Per core:
  - q/k projections in [e, s] layout (weights stationary, x moving, N=512)
  - v projection directly in NATURAL [s, e] layout (x stationary, w moving)
    so no DRAM-round-trip transpose is needed; bias folded in via a
    row-replicated bias tile (tensor_tensor add during the PSUM drain).
  - attention in transposed form: scores^T[j, i] per 128-j tile, softmax-exp
    on ACT reading PSUM directly (scale=1/8 fused), fp16 everywhere on chip.
  - PV with exp STATIONARY and v moving (out [i-block, 65] at N=65), which
    charges ~half the PE rows of the v-stationary form; a ones column rides
    in v to produce sumexp per position in the same matmuls.
  - per-position Gram-Schmidt exclusion entirely with per-partition scalars
    (i on partitions): reductions along the free dim on DVE, no PE
    broadcasts needed. EPS dropped: vv in [33, 180] makes it irrelevant.
  - o_f transposed back [i,e]->[e,i] via PE transpose (identity passed as an
    input), then the partial out-projection; partials fp16, summed on host.

PSUM discipline (device-verified): a bank supports only ONE open matmul
accumulation group at a time and groups must be contiguous runs, so each
PV (h, i-block) is an unbroken 16-matmul burst, chained with explicit deps
so the tile scheduler cannot interleave same-bank groups. Bank budget (8):
scores ping-pong 2x[128,1024] (4) + pv h0/h1 (2) + outproj/transpose/proj
scratch (2).

Software-pipelined emission: rounds of (batch, i-quarter); per score-tile
the PE stream carries "filler" quanta (projections of the other batch, PV
bursts and out-projection of the previous round) via a slotted work queue,
so the PE stays fed while ACT runs the exp stream (the ~133us ACT exp
budget is the binding engine). The kernel tail routes PSUM drains to ACT,
which is idle once the exp stream ends.
"""

import sys

if '/opt/trn_rl_repo' not in sys.path:
    sys.path.insert(0, '/opt/trn_rl_repo')

import numpy as np
import ml_dtypes

import concourse.bass as bass
import concourse.mybir as mybir
import concourse.tile as tile
from concourse.bass_utils import run_bass_kernel_spmd

F32 = mybir.dt.float32
F16 = mybir.dt.float16
AF = mybir.ActivationFunctionType
ALU = mybir.AluOpType

B, S, D = 2, 2048, 1024
BS = B * S                    # 4096 combined (b, s) rows
E_LOC = 128                   # channels per core (2 heads x 64)
N_CORES = 8
INV_SQRT_HD = 0.125
NT = 32                       # 128-row sequence tiles
NR = 8                        # rounds: (b, i-quarter), i extent 512 each
IS = 512

_ENGINE_TO_NC = {"PE": "tensor", "DVE": "vector", "Activation": "scalar",
                 "Pool": "gpsimd", "SP": "sync"}


def _make_nop(nc, engine):
    eng = getattr(nc, _ENGINE_TO_NC[str(engine).split(".")[-1]])
    r = eng.nop(nofuse=True, hint="waitsplit")
    ins = r.ins if hasattr(r, "ins") else r
    for blk in nc.main_func.blocks:
        insns = blk.instructions
        for i, x in enumerate(insns):
            if x.name == ins.name:
                del insns[i]
                blk.instructions = insns
                return ins
    raise RuntimeError("freshly created nop not found")


def split_waits(nc, limit=1):
    """Walrus codegen only encodes one sync-wait per instruction here; move
    excess waits onto preceding same-engine NOPs (same-engine program order
    makes this semantics-preserving)."""
    for blk in nc.main_func.blocks:
        ins_list = blk.instructions
        out, changed = [], False
        for ins in ins_list:
            si = ins.sync_info
            if si is not None and len(si.on_wait) > limit:
                waits = list(si.on_wait)
                extra, keep = waits[:-limit], waits[-limit:]
                for w in extra:
                    nop = _make_nop(nc, ins.engine)
                    nop.sync_info = mybir.SyncInfo(on_wait=[w], on_update=[])
                    out.append(nop)
                ins.sync_info = mybir.SyncInfo(on_wait=keep, on_update=list(si.on_update))
                changed = True
            out.append(ins)
        if changed:
            blk.instructions = out


def build_program():
    nc = bass.Bass()

    xT_d = nc.declare_dram_parameter("xT", [D, BS], F16, isOutput=False)
    wqT_d = nc.declare_dram_parameter("wqT", [D, E_LOC], F16, isOutput=False)
    wkT_d = nc.declare_dram_parameter("wkT", [D, E_LOC], F16, isOutput=False)
    wvT_d = nc.declare_dram_parameter("wvT", [D, E_LOC], F16, isOutput=False)
    bq_d = nc.declare_dram_parameter("bq", [E_LOC], F32, isOutput=False)
    bk_d = nc.declare_dram_parameter("bk", [E_LOC], F32, isOutput=False)
    bvr_d = nc.declare_dram_parameter("bvr", [128, E_LOC], F16, isOutput=False)
    id_d = nc.declare_dram_parameter("ident", [128, 128], F16, isOutput=False)
    woT_d = nc.declare_dram_parameter("woT", [E_LOC, D], F16, isOutput=False)
    part_d = nc.declare_dram_parameter("partial", [BS, D], F16, isOutput=True)

    with tile.TileContext(nc) as tc:
        import contextlib
        with contextlib.ExitStack() as ctx:
            const = ctx.enter_context(tc.tile_pool(name="const", bufs=1))
            xpool = ctx.enter_context(tc.tile_pool(name="xpool", bufs=1))
            persist = ctx.enter_context(tc.tile_pool(name="persist", bufs=1))
            etp = ctx.enter_context(tc.tile_pool(name="etp", bufs=26))
            ofp = ctx.enter_context(tc.tile_pool(name="ofp", bufs=24))
            oftp = ctx.enter_context(tc.tile_pool(name="oftp", bufs=4))
            ystp = ctx.enter_context(tc.tile_pool(name="ystp", bufs=4))
            scrp = ctx.enter_context(tc.tile_pool(name="scrp", bufs=2))
            ovp = ctx.enter_context(tc.tile_pool(name="ovp", bufs=2))
            alp = ctx.enter_context(tc.tile_pool(name="alp", bufs=4))
            rzp = ctx.enter_context(tc.tile_pool(name="rzp", bufs=4))
            avrp = ctx.enter_context(tc.tile_pool(name="avrp", bufs=8))
            vvrp = ctx.enter_context(tc.tile_pool(name="vvrp", bufs=2))
            ps_sc = ctx.enter_context(tc.tile_pool(name="ps_sc", bufs=2, space="PSUM"))
            ps_pv = ctx.enter_context(tc.tile_pool(name="ps_pv", bufs=2, space="PSUM"))
            ps_op = ctx.enter_context(tc.tile_pool(name="ps_op", bufs=2, space="PSUM"))

            # ---- constants (DMA order tuned for early k-projection) ----
            xt = xpool.tile([128, 8, BS], F16, tag="xt")

            def dma_x(c, kts=slice(0, 8)):
                nc.sync.dma_start(
                    out=xt[:, kts, c * 512:(c + 1) * 512],
                    in_=xT_d[:, c * 512:(c + 1) * 512].rearrange(
                        "(kt p) s -> p kt s", kt=8)[:, kts, :])

            def dma_x_cols(c0, c1):
                nc.sync.dma_start(
                    out=xt[:, :, c0:c1],
                    in_=xT_d[:, c0:c1].rearrange("(kt p) s -> p kt s", kt=8))

            # DMA order: the critical path to the first exp is
            # ident -> wk -> x[0:256] -> wq -> x[256:512] -> biases
            id_sb = const.tile([128, 128], F16, tag="ident")
            nc.sync.dma_start(out=id_sb, in_=id_d[:, :])
            wk_sb = const.tile([128, 8, E_LOC], F16, tag="wk")
            nc.sync.dma_start(out=wk_sb, in_=wkT_d[:, :].rearrange(
                "(kt p) e -> p kt e", kt=8))
            dma_x_cols(0, 256)
            wq_sb = const.tile([128, 8, E_LOC], F16, tag="wq")
            nc.sync.dma_start(out=wq_sb, in_=wqT_d[:, :].rearrange(
                "(kt p) e -> p kt e", kt=8))
            dma_x_cols(256, 512)
            bk_sb = const.tile([128, 1], F32, tag="bk")
            nc.sync.dma_start(out=bk_sb, in_=bk_d[:].rearrange("(p one) -> p one", one=1))
            bq_sb = const.tile([128, 1], F32, tag="bq")
            nc.sync.dma_start(out=bq_sb, in_=bq_d[:].rearrange("(p one) -> p one", one=1))
            wv_sb = const.tile([128, 8, E_LOC], F16, tag="wv")
            nc.sync.dma_start(out=wv_sb, in_=wvT_d[:, :].rearrange(
                "(kt p) e -> p kt e", kt=8))
            bvr_sb = const.tile([128, E_LOC], F16, tag="bvr")
            nc.sync.dma_start(out=bvr_sb, in_=bvr_d[:, :])
            dma_x(1)
            wo_sb = const.tile([128, D], F16, tag="wo")
            nc.sync.dma_start(out=wo_sb, in_=woT_d[:, :])
            for c in range(2, 8):
                dma_x(c)

            wsb = {"q": wq_sb, "k": wk_sb, "v": wv_sb}

            # ---- persistent activations ----
            qT = persist.tile([128, BS], F16, tag="qT")
            kT = persist.tile([128, BS], F16, tag="kT")
            # vn: cols 0:64 v_h0, 64 ones, 65:129 v_h1, 129 pad
            vn = persist.tile([128, NT, 130], F16, tag="vn")
            vv = persist.tile([128, 2, NT], F32, tag="vv")
            nc.vector.memset(vn[:, :, 64:65], 1.0)

            # ---- projection work quanta ----
            _kq_ps = {}

            def kq_piece(which, c0, c1, kta, ktb):
                """q/k projection of s-columns [c0, c1), kt-tiles [kta, ktb)."""
                key = (which, c0)
                if kta == 0:
                    _kq_ps[key] = ps_op.tile([128, 512], F32, tag="op",
                                             name=f"ps{which}{c0}")
                ps = _kq_ps[key]
                for kt in range(kta, ktb):
                    nc.tensor.matmul(ps[:, 0:c1 - c0], wsb[which][:, kt, :],
                                     xt[:, kt, c0:c1],
                                     start=(kt == 0), stop=(kt == 7))
                if ktb == 8:
                    dst, bias = (qT, bq_sb) if which == "q" else (kT, bk_sb)
                    nc.vector.tensor_scalar(
                        out=dst[:, c0:c1], in0=ps[:, 0:c1 - c0],
                        scalar1=bias, scalar2=None, op0=ALU.add)
                    del _kq_ps[key]

            def kq_half(which, c, half):
                kq_piece(which, c * 512, (c + 1) * 512, 4 * half, 4 * half + 4)

            def v_tile(t):
                """v projection for sequence tile t, into natural layout."""
                ps = ps_op.tile([128, 512], F32, tag="op", name=f"psv{t}")
                for kt in range(8):
                    nc.tensor.matmul(ps[:, 0:128],
                                     xt[:, kt, t * 128:(t + 1) * 128],
                                     wsb["v"][:, kt, :],
                                     start=(kt == 0), stop=(kt == 7))
                nc.vector.tensor_tensor(out=vn[:, t, 0:64], in0=ps[:, 0:64],
                                        in1=bvr_sb[:, 0:64], op=ALU.add)
                nc.vector.tensor_tensor(out=vn[:, t, 65:129], in0=ps[:, 64:128],
                                        in1=bvr_sb[:, 64:128], op=ALU.add)
                for h in range(2):
                    hs = slice(65 * h, 65 * h + 64)
                    scr = scrp.tile([128, 64], F16, tag="scr")
                    nc.vector.scalar_tensor_tensor(
                        out=scr, in0=vn[:, t, hs], scalar=1.0,
                        in1=vn[:, t, hs], op0=ALU.mult, op1=ALU.mult,
                        accum_out=vv[:, h, t:t + 1])

            # ---- attention round machinery ----
            from concourse.tile import add_dep_helper

            def _ins(x):
                return x.ins if hasattr(x, "ins") else x

            def pv_burst(pv, ets, jb, h, m, prev_last):
                """One contiguous 16-matmul accumulation group (h, i-block m).
                PSUM allows only one open accumulation group per bank, so the
                group must run as an unbroken run on its bank; chain it after
                the previous group of the same bank."""
                first = last = None
                for jt in range(16):
                    mvs = vn[:, jb + jt, 0:65] if h == 0 else vn[:, jb + jt, 64:129]
                    r = nc.tensor.matmul(
                        pv[:, 65 * m:65 * m + 65],
                        ets[jt][:, h * 512 + m * 128:h * 512 + (m + 1) * 128],
                        mvs,
                        start=(jt == 0), stop=(jt == 15),
                        skip_group_check=True)
                    if jt == 0:
                        first = r
                    last = r
                if prev_last is not None:
                    add_dep_helper(_ins(first), _ins(prev_last),
                                   reason="psum accumulation groups must not "
                                          "interleave within a bank")
                return last

            ZOFF = {0: 64, 1: 0}   # Z column offset within pv slot
            OOFF = {0: 0, 1: 1}    # o columns offset within pv slot

            def exclusion_m(r, pv, vvr_r, t0, m):
                """Per-i-block Gram-Schmidt exclusion (both heads)."""
                of = ofp.tile([128, 128], F16, tag="of", name=f"of{r}{m}")
                for h in range(2):
                    pvh = pv[h]
                    hs = slice(65 * h, 65 * h + 64)
                    rz = rzp.tile([128, 1], F32, tag="rz")
                    nc.vector.reciprocal(
                        rz, pvh[:, 65 * m + ZOFF[h]:65 * m + ZOFF[h] + 1])
                    ov = ovp.tile([128, 1], F32, tag="ov")
                    scr = scrp.tile([128, 64], F16, tag="scr")
                    nc.vector.scalar_tensor_tensor(
                        out=scr,
                        in0=pvh[:, 65 * m + OOFF[h]:65 * m + OOFF[h] + 64],
                        scalar=1.0,
                        in1=vn[:, t0 + m, hs],
                        op0=ALU.mult, op1=ALU.mult,
                        accum_out=ov)
                    al = alp.tile([128, 1], F32, tag="al")
                    nc.vector.tensor_tensor(out=al, in0=ov,
                                            in1=vvr_r[:, h, m:m + 1],
                                            op=ALU.mult)
                    alr = alp.tile([128, 1], F32, tag="al", name=f"alr{r}{h}{m}")
                    nc.vector.tensor_tensor(out=alr, in0=al, in1=rz, op=ALU.mult)
                    avr = avrp.tile([128, 64], F16, tag="avr")
                    nc.gpsimd.tensor_scalar(
                        out=avr, in0=vn[:, t0 + m, hs],
                        scalar1=alr, scalar2=None, op0=ALU.mult)
                    nc.vector.scalar_tensor_tensor(
                        out=of[:, 64 * h:64 * h + 64],
                        in0=pvh[:, 65 * m + OOFF[h]:65 * m + OOFF[h] + 64],
                        scalar=rz, in1=avr,
                        op0=ALU.mult, op1=ALU.subtract)
                return of

            def outproj_steps_m(r, of, m, use_act=False):
                """3 filler quanta: transpose + 2 half-d out-proj matmuls for
                one i-block. use_act routes PSUM drains to ACT (kernel tail,
                where the exp stream is finished)."""
                oft_box = {}
                cp = nc.scalar.copy if use_act else nc.vector.tensor_copy

                def tr():
                    tp = ps_op.tile([128, 128], F16, tag="op",
                                    padded_shape=[128, 1024],
                                    name=f"tp{r}{m}")
                    nc.tensor.transpose(tp, of, id_sb)
                    oft = oftp.tile([128, 128], F16, tag="oft")
                    cp(oft, tp)
                    oft_box[0] = oft

                def ymm(dh):
                    def f():
                        ps = ps_op.tile([128, 512], F32, tag="op",
                                        name=f"y{r}{m}{dh}")
                        nc.tensor.matmul(ps, oft_box[0],
                                         wo_sb[:, dh * 512:(dh + 1) * 512],
                                         start=True, stop=True)
                        stg = ystp.tile([128, 512], F16, tag="ystg")
                        cp(stg, ps)
                        ig = r * 512 + m * 128
                        nc.sync.dma_start(
                            out=part_d[ig:ig + 128, dh * 512:(dh + 1) * 512],
                            in_=stg)
                    return f

                return [tr, ymm(0), ymm(1)]

            # ---- work queues: (earliest_global_slot, cycles, closure) ----
            # deferq holds work with no deadline (early rounds' out-proj),
            # drained only when the main queue is empty for this slot
            import collections
            workq = collections.deque()
            deferq = collections.deque()
            gslot_box = [0]

            def run_queue(budget):
                while budget > 0:
                    if workq and workq[0][0] <= gslot_box[0]:
                        q = workq
                    elif deferq and deferq[0][0] <= gslot_box[0]:
                        q = deferq
                    else:
                        break
                    _, cyc, f = q.popleft()
                    f()
                    budget -= cyc
                return budget

            def enqueue(earliest, cyc, f, defer=False):
                (deferq if defer else workq).append((earliest, cyc, f))

            def pv_and_excl(r):
                """Enqueue PV bursts of round r into round r+1's slots; after
                both heads' burst of i-block m, its exclusion runs and the
                block's out-projection is enqueued. The last round starts two
                slots early (its PE stream has nothing else left)."""
                b = r // 4
                jb = b * 16
                t0 = r * 4
                last = (r == NR - 1)
                pv = {h: ps_pv.tile([128, 512], F32, tag="pv",
                                    name=f"pv{r}{h}") for h in range(2)}
                ets = ets_of[r]
                state = {0: None, 1: None}
                vvr_box = {}
                base = r * 16 + 14 if last else (r + 1) * 16

                def burst(h, m):
                    def f():
                        if not vvr_box:
                            vvr_r = vvrp.tile([128, 2, 4], F32, tag="vvr")
                            nc.vector.reciprocal(vvr_r, vv[:, :, t0:t0 + 4])
                            vvr_box[0] = vvr_r
                        state[h] = pv_burst(pv[h], ets, jb, h, m, state[h])
                        if h == 1:
                            of = exclusion_m(r, pv, vvr_box[0], t0, m)
                            if m == 3:
                                del ets_of[r]
                            sts = outproj_steps_m(r, of, m, use_act=last)
                            for i, st in enumerate(sts):
                                enqueue(base + 2 * m + 2 + i, 600, st,
                                        defer=(r <= 2))
                    return f

                for m in range(4):
                    enqueue(base + 2 * m, 1040, burst(0, m))
                    enqueue(base + 2 * m + 1, 1040, burst(1, m))

            ets_of = {}

            def round_(r, fillers):
                b, qi = divmod(r, 4)
                i0 = b * S + qi * IS
                jb = b * 16
                ets = {}
                ets_of[r] = ets
                for jt in range(16):
                    gslot_box[0] = r * 16 + jt
                    sc = ps_sc.tile([128, 1024], F32, tag="sc")
                    jcol = slice((jb + jt) * 128, (jb + jt + 1) * 128)
                    nc.tensor.matmul(sc[:, 0:512], kT[0:64, jcol],
                                     qT[0:64, i0:i0 + IS],
                                     start=True, stop=True, tile_position=(0, 0))
                    nc.tensor.matmul(sc[:, 512:1024], kT[64:128, jcol],
                                     qT[64:128, i0:i0 + IS],
                                     start=True, stop=True, tile_position=(64, 0))
                    et = etp.tile([128, 1024], F16, tag="et")
                    nc.scalar.activation(et, sc, AF.Exp, bias=0.0,
                                         scale=INV_SQRT_HD)
                    ets[jt] = et
                    budget = 1500
                    for cyc, f in fillers[jt]:
                        f()
                        budget -= cyc
                    run_queue(budget)
                pv_and_excl(r)

            # ---- emission schedule ----
            # warm up the PE p-state during the initial DMA wait: back-to-back
            # junk matmuls (WAW-serialized on one psum slot) keep the PE busy
            # >3us so the first real projection runs at full clock
            warm_ps = ps_op.tile([128, 512], F32, tag="op", name="warm")
            for _ in range(10):
                nc.tensor.matmul(warm_ps[:, 0:128], id_sb, id_sb,
                                 start=True, stop=True)
            # pre-phase: the minimum for scores jt0/jt1 — k of s[0:256] and
            # q of i[0:512]; the rest of k chunk 0 is the first round-0 filler
            kq_piece("k", 0, 256, 0, 8)
            kq_half("q", 0, 0)
            kq_half("q", 0, 1)

            def empty_sched():
                return [[] for _ in range(16)]

            def F_kq(which, c, half):
                return (2048, lambda: kq_half(which, c, half))

            def F_v(t):
                return (1024, lambda: v_tile(t))

            for r in range(NR):
                fill = empty_sched()
                if r == 0:
                    # b0 k chunks (deadline: scores jt 4c), v tiles 4..15
                    # (deadline: pv bursts next round), q(b0,i1) late
                    fill[0] += [(2048, lambda: kq_piece("k", 256, 512, 0, 8))]
                    fill[1] += [F_kq("k", 1, 0)]
                    fill[2] += [F_kq("k", 1, 1)]
                    fill[3] += [F_v(0), F_v(1)]
                    fill[4] += [F_kq("k", 2, 0)]
                    fill[5] += [F_kq("k", 2, 1)]
                    fill[6] += [F_v(2), F_v(3)]
                    fill[7] += [F_v(4), F_v(5)]
                    fill[8] += [F_kq("k", 3, 0)]
                    fill[9] += [F_kq("k", 3, 1)]
                    fill[10] += [F_v(6), F_v(7)]
                    fill[11] += [F_v(8), F_v(9)]
                    fill[12] += [F_v(10), F_v(11)]
                    fill[13] += [F_v(12), F_v(13), F_kq("q", 1, 0)]
                    fill[15] += [F_v(14), F_v(15)]
                    fill[14] += [F_kq("q", 1, 1)]
                elif r in (1, 2):
                    qc = r + 1          # q(b0, i2) in r1, q(b0, i3) in r2
                    kc = 4 + 2 * (r - 1)  # k(b1) chunks 4,5 in r1; 6,7 in r2
                    fill[9] += [F_kq("q", qc, 0)]
                    fill[10] += [F_kq("q", qc, 1)]
                    fill[11] += [F_kq("k", kc, 0)]
                    fill[12] += [F_kq("k", kc, 1)]
                    fill[13] += [F_kq("k", kc + 1, 0)]
                    fill[14] += [F_kq("k", kc + 1, 1)]
                elif r == 3:
                    fill[8] += [F_kq("q", 4, 0)]
                    fill[9] += [F_kq("q", 4, 1)]
                    for i, t in enumerate(range(16, 26)):
                        fill[10 + (i * 5) // 10] += [F_v(t)]
                elif r == 4:
                    # remaining b1 v tiles (needed by pv bursts in round 5)
                    fill[8] += [F_kq("q", 5, 0)]
                    fill[9] += [F_kq("q", 5, 1)]
                    for i, t in enumerate(range(26, 32)):
                        fill[10 + (i * 5) // 10] += [F_v(t)]
                elif r in (5, 6):
                    qc = r + 1
                    fill[8] += [F_kq("q", qc, 0)]
                    fill[10] += [F_kq("q", qc, 1)]
                round_(r, fill)

            # tail: drain remaining queued work (last rounds' PV/excl/outproj)
            gslot_box[0] = 10 ** 9
            while workq:
                _, _, f = workq.popleft()
                f()

    split_waits(nc)
    return nc


_CACHE = {}


def kernel(x, wq, bq, wk, bk, wv, bv, wo, bo):
    x = np.ascontiguousarray(np.asarray(x, dtype=np.float32))
    wq, wk, wv, wo = (np.asarray(w, dtype=np.float32) for w in (wq, wk, wv, wo))
    bq, bk, bv, bo = (np.asarray(v, dtype=np.float32) for v in (bq, bk, bv, bo))

    if "nc" not in _CACHE:
        _CACHE["nc"] = build_program()
    nc = _CACHE["nc"]

    xT = np.ascontiguousarray(x.reshape(BS, D).T).astype(np.float16)
    ident = np.eye(128, dtype=np.float16)
    in_maps = []
    for g in range(N_CORES):
        cs = slice(g * E_LOC, (g + 1) * E_LOC)
        in_maps.append({
            "xT": xT,
            "wqT": np.ascontiguousarray(wq[cs, :].T).astype(np.float16),
            "wkT": np.ascontiguousarray(wk[cs, :].T).astype(np.float16),
            "wvT": np.ascontiguousarray(wv[cs, :].T).astype(np.float16),
            "bq": np.ascontiguousarray(bq[cs]),
            "bk": np.ascontiguousarray(bk[cs]),
            "bvr": np.ascontiguousarray(
                np.tile(bv[cs].astype(np.float16)[None, :], (128, 1))),
            "ident": ident,
            "woT": np.ascontiguousarray(wo[:, cs].T).astype(np.float16),
        })

    res = run_bass_kernel_spmd(nc, in_maps, list(range(N_CORES)))
    out = np.zeros((BS, D), np.float32)
    for g in range(N_CORES):
        out += np.asarray(res.results[g]["partial"], np.float32)
    out += bo[None, :]
    return out.reshape(B, S, D)
